# revision 1
# baseline (speedup 1.0000x reference)
"""NanoVLM GQA attention block on 8 Trainium2 NeuronCores.

Sharding: core c = 4*b + g handles batch b (of 2) and head-group g (of 4):
8 q-heads (global 8g..8g+8) and 2 kv-heads (2g, 2g+1). Each core computes a
partial output restricted to its heads' columns of Wo; the host sums the 4
partials per batch (the tensor-parallel reduce, done host-side).

Device pipeline (per core, fp32r matmuls = full PE rate, ~1e-4 rounding):
  1. proj with WEIGHTS stationary -> q/k/v directly d-major [hd, t] in PSUM
     (q as 4 head-pair tiles of 128 partitions, k as one kv-pair, v pair)
  2. RoPE in d-major: rotate_half = partition 32<->64 block swap done with
     SBUF->SBUF DMAs (sign folded into the sin table); cos/sin tables are
     [d, t] and shared by all tiles; 1/sqrt(64) folded into q's tables
  3. v transposed back to t-major [t, hd] (PV needs k on partitions),
     with an appended ones column -> v_aug [128, 65]
  4. scores k-major: sp [128k, 512q] = kT_chunk^T @ qT slice (operands use
     partition offset 64 for odd heads), causal sub-ranges only; diagonal
     128-block gets an additive mask; exp on ACT with per-partition bias
     = gate[h, 2vq+vk] + log-mask(attention_mask), writing P^T f32r
  5. PV: y_aug [65, 512q] += v_aug[kc]^T @ P^T  (row 64 = softmax denom l)
  6. normalize: PE-transpose y_aug blocks -> [128q, 65], reciprocal of
     col 64, tensor_scalar multiply -> y [t, hd]
  7. PE-transpose y -> yT [hd, t], out-proj psum [128t, 512n] over 4
     hd-pair chunks, evac, DMA partial out [1024, 2048]
"""

import os
import sys

sys.path.insert(0, "/opt/trn_rl_repo")

import numpy as np

import concourse.bacc as bacc
import concourse.mybir as mybir
import concourse.tile as tile
from concourse.bass_utils import run_bass_kernel_spmd
from concourse.masks import make_identity

F32 = mybir.dt.float32
F32R = mybir.dt.float32r
AF = mybir.ActivationFunctionType
ALU = mybir.AluOpType

B, T, C = 2, 1024, 2048
NH, NKV, HD = 32, 8, 64
QH, KVH = 8, 2          # per-core q-heads / kv-heads
NTB = T // 128          # 8 t-blocks
NCORES = 8
NEG = -1e30


def build_program(qtile_vq):
    """qtile_vq: per 128-token q-tile, the is_vision value (0/1), len 8."""
    nc = bacc.Bacc("TRN2", target_bir_lowering=False, debug=False,
                   num_devices=NCORES)

    xT_d = nc.dram_tensor("xT", [C, T], F32, kind="ExternalInput").ap()
    wq_d = nc.dram_tensor("wqT", [C, 512], F32, kind="ExternalInput").ap()
    wkv_d = nc.dram_tensor("wkvT", [C, 256], F32, kind="ExternalInput").ap()
    wo_d = nc.dram_tensor("woT", [512, C], F32, kind="ExternalInput").ap()
    cosq_d = nc.dram_tensor("cosqT", [128, T], F32, kind="ExternalInput").ap()
    sinq_d = nc.dram_tensor("sinqT", [128, T], F32, kind="ExternalInput").ap()
    cosk_d = nc.dram_tensor("coskT", [128, T], F32, kind="ExternalInput").ap()
    sink_d = nc.dram_tensor("sinkT", [128, T], F32, kind="ExternalInput").ap()
    btab_d = nc.dram_tensor("btab", [128, 128], F32, kind="ExternalInput").ap()
    maskT_d = nc.dram_tensor("maskT", [128, 128], F32, kind="ExternalInput").ap()
    ones_d = nc.dram_tensor("ones", [128, 1], F32, kind="ExternalInput").ap()
    out_d = nc.dram_tensor("out", [T, C], F32, kind="ExternalOutput").ap()

    with tile.TileContext(nc) as tc:
        cp_cm = tc.tile_pool(name="const", bufs=1)
        cp = cp_cm.__enter__()
        ident = cp.tile([128, 128], F32, tag="ident")
        make_identity(nc, ident[:])
        btab = cp.tile([128, 128], F32, tag="btab")
        nc.scalar.dma_start(btab[:], btab_d)
        maskT = cp.tile([128, 128], F32, tag="maskT")
        nc.scalar.dma_start(maskT[:], maskT_d)
        qTp = [cp.tile([128, T], F32R, tag=f"qTp{p}", name=f"qTp{p}")
               for p in range(4)]
        kTp = cp.tile([128, T], F32R, tag="kTp")
        kTs = cp.tile([128, T], F32R, tag="kTs")  # kv heads swapped
        vA = [[cp.tile([128, 65], F32R, tag=f"v{tb}_{j}", name=f"v{tb}_{j}")
               for j in range(KVH)] for tb in range(NTB)]
        for tb in range(NTB):
            for j in range(KVH):
                nc.gpsimd.dma_start(vA[tb][j][:, 64:65], ones_d.bitcast(F32R))
        p2y_cm = tc.tile_pool(name="p2y", bufs=1)
        p2y = p2y_cm.__enter__()
        ysb = [p2y.tile([128, 512], F32, tag=f"y{tb}", name=f"y{tb}")
               for tb in range(NTB)]

        # --------- phase-1 pools (th0 x + th0 tables / weights / tmps) ----
        p1w_cm = tc.tile_pool(name="p1w", bufs=1, side="right")
        p1w = p1w_cm.__enter__()
        p1t_cm = tc.tile_pool(name="p1t", bufs=2, side="right")
        p1t = p1t_cm.__enter__()
        p1xa_cm = tc.tile_pool(name="p1xa", bufs=1, side="right")
        p1xa = p1xa_cm.__enter__()
        p1ps_cm = tc.tile_pool(name="p1ps", bufs=1, space="PSUM")
        p1ps = p1ps_cm.__enter__()
        p1pv_cm = tc.tile_pool(name="p1pv", bufs=2, space="PSUM")
        p1pv = p1pv_cm.__enter__()

        xa, wqs, wkvs = [], [], []
        for i in range(16):
            xt = p1xa.tile([128, 512], F32R, tag=f"xa{i}", name=f"xa{i}")
            nc.sync.dma_start(xt[:], xT_d[i * 128:(i + 1) * 128, 0:512].bitcast(F32R))
            xa.append(xt)
            wt = p1w.tile([128, 512], F32R, tag=f"wq{i}", name=f"wq{i}")
            nc.gpsimd.dma_start(wt[:], wq_d[i * 128:(i + 1) * 128, :].bitcast(F32R))
            wqs.append(wt)
            kt = p1w.tile([128, 256], F32R, tag=f"wkv{i}", name=f"wkv{i}")
            nc.scalar.dma_start(kt[:], wkv_d[i * 128:(i + 1) * 128, :].bitcast(F32R))
            wkvs.append(kt)
        tab_a = {}
        for nm, dr in (("cq", cosq_d), ("sq", sinq_d), ("ck", cosk_d), ("sk", sink_d)):
            ta = p1xa.tile([128, 512], F32, tag=f"{nm}a", name=f"{nm}a")
            nc.scalar.dma_start(ta[:], dr[:, 0:512])
            tab_a[nm] = ta

        def rope_blk(pp, blk, th, tabs):
            """pp: [128,512] psum with d-major proj; writes qTp/kTp th-slice."""
            tsl = slice(th * 512, (th + 1) * 512)
            cosT = tabs["cq"] if blk < 4 else tabs["ck"]
            sinT = tabs["sq"] if blk < 4 else tabs["sk"]
            dstT = qTp[blk] if blk < 4 else kTp
            ev = p1t.tile([128, 512], F32, tag="ev", name="ev")
            nc.scalar.copy(ev[:], pp[:])
            rot = p1t.tile([128, 512], F32, tag="rot", name="rot")
            for q0 in (0, 64):
                nc.gpsimd.dma_start(rot[q0:q0 + 32, :], ev[q0 + 32:q0 + 64, :])
                nc.gpsimd.dma_start(rot[q0 + 32:q0 + 64, :], ev[q0:q0 + 32, :])
            t1 = p1t.tile([128, 512], F32, tag="t1", name="t1")
            nc.vector.tensor_mul(t1[:], pp[:], cosT[:])
            t2 = p1t.tile([128, 512], F32, tag="t2", name="t2")
            nc.vector.tensor_mul(t2[:], rot[:], sinT[:])
            nc.vector.tensor_add(dstT[:, tsl], t1[:], t2[:])
            if blk == 4:
                nc.gpsimd.dma_start(kTs[0:64, tsl], kTp[64:128, tsl])
                nc.gpsimd.dma_start(kTs[64:128, tsl], kTp[0:64, tsl])

        def v_evac(pp):
            vsb = p1t.tile([128, 512], F32, tag="vsb", name="vsb")
            nc.scalar.copy(vsb[:], pp[:])
            return vsb

        def v_transposes(th, vsb, pool):
            for qb in range(4):
                tb = th * 4 + qb
                vt = pool.tile([128, 128], F32, tag=pool._vt_tag, name="vt")
                nc.tensor.transpose(vt[:], vsb[:, qb * 128:(qb + 1) * 128],
                                    ident[:])
                nc.vector.tensor_copy(vA[tb][0][:, 0:64], vt[:, 0:64])
                nc.vector.tensor_copy(vA[tb][1][:, 0:64], vt[:, 64:128])

        # --------- th0 projection: ci-outer over 6 psum blocks ------------
        pps = [p1ps.tile([128, 512], F32, tag=f"pp{b}", name=f"pp{b}")
               for b in range(6)]
        for ci in range(16):
            for blk in (4, 0):
                lhsT = (wqs[ci][:, 0:128] if blk == 0
                        else wkvs[ci][:, 0:128])
                nc.tensor.matmul(pps[blk][:], lhsT, xa[ci][:],
                                 start=(ci == 0), stop=(ci == 15))
        rope_blk(pps[4], 4, 0, tab_a)
        rope_blk(pps[0], 0, 0, tab_a)
        for ci in range(16):
            for blk in (1, 2, 3, 5):
                if blk < 4:
                    lhsT = wqs[ci][:, blk * 128:(blk + 1) * 128]
                else:
                    lhsT = wkvs[ci][:, 128:256]
                nc.tensor.matmul(pps[blk][:], lhsT, xa[ci][:],
                                 start=(ci == 0), stop=(ci == 15))
        for blk in (1, 2, 3):
            rope_blk(pps[blk], blk, 0, tab_a)
        vsb0 = v_evac(pps[5])
        p1pv._vt_tag = "vt"
        v_transposes(0, vsb0, p1pv)

        p1pv_cm.__exit__(None, None, None)
        p1ps_cm.__exit__(None, None, None)
        p1xa_cm.__exit__(None, None, None)

        # --------- attention pools (+ th1 x / tables, DMA'd now) ----------
        ptp_cm = tc.tile_pool(name="ptp", bufs=8)
        ptp = ptp_cm.__enter__()
        p2t_cm = tc.tile_pool(name="p2t", bufs=2)
        p2t = p2t_cm.__enter__()
        p1xb_cm = tc.tile_pool(name="p1xb", bufs=1, side="right")
        p1xb = p1xb_cm.__enter__()
        psA_cm = tc.tile_pool(name="psA", bufs=4, space="PSUM")
        psA = psA_cm.__enter__()
        psB_cm = tc.tile_pool(name="psB", bufs=1, space="PSUM")
        psB = psB_cm.__enter__()
        psC_cm = tc.tile_pool(name="psC", bufs=2, space="PSUM")
        psC = psC_cm.__enter__()
        psC._vt_tag = "tp2"

        xb = []
        for i in range(16):
            xt = p1xb.tile([128, 512], F32R, tag=f"xb{i}", name=f"xb{i}")
            nc.sync.dma_start(xt[:], xT_d[i * 128:(i + 1) * 128, 512:1024].bitcast(F32R))
            xb.append(xt)
        tab_b = {}
        for nm, dr in (("cq", cosq_d), ("sq", sinq_d), ("ck", cosk_d), ("sk", sink_d)):
            ta = p1xb.tile([128, 512], F32, tag=f"{nm}b", name=f"{nm}b")
            nc.scalar.dma_start(ta[:], dr[:, 512:1024])
            tab_b[nm] = ta

        def scores(s, h, kc, pts):
            j, p, r = h // 4, h // 2, (h % 2) * 64
            kt = kTp if j * 64 == r else kTs
            ql = max(0, kc * 128 - s * 512)
            sp = psA.tile([128, 512], F32, tag="sp", name="sp")
            nc.tensor.matmul(
                sp[:, ql:512],
                kt[r:r + 64, kc * 128:(kc + 1) * 128],
                qTp[p][r:r + 64, s * 512 + ql:(s + 1) * 512],
                start=True, stop=True)
            if s * 4 <= kc < s * 4 + 4:
                nc.vector.scalar_tensor_tensor(
                    sp[:, ql:ql + 128], sp[:, ql:ql + 128],
                    0.0, maskT[:], op0=ALU.add, op1=ALU.add)
            pt = ptp.tile([128, 512], F32R, tag="pt", name="pt")
            c = ql  # multiple of 128
            while c < 512:
                vq = qtile_vq[s * 4 + c // 128]
                ce = c
                while ce < 512 and qtile_vq[s * 4 + ce // 128] == vq:
                    ce += 128
                col = h * 16 + vq * 8 + kc
                nc.scalar.activation(pt[:, c:ce], sp[:, c:ce], AF.Exp,
                                     bias=btab[:, col:col + 1], scale=1.0)
                c = ce
            pts[kc] = pt

        def pv(s, h, kc, kcmax, yp, pts):
            j = h // 4
            ql = max(0, kc * 128 - s * 512)
            nc.tensor.matmul(
                yp[:, ql:512], vA[kc][j][:, 0:65], pts[kc][:, ql:512],
                start=(kc == 0), stop=(kc == kcmax - 1),
                skip_group_check=True)
            pts[kc] = None

        def normalize(s, h, yp):
            ya = p2t.tile([65, 512], F32, tag="ya", name="ya")
            nc.scalar.copy(ya[:], yp[:])
            for qb in range(4):
                tq = psC.tile([128, 65], F32, tag="tp2", name="tq")
                nc.tensor.transpose(tq[:], ya[:, qb * 128:(qb + 1) * 128],
                                    ident[0:65, 0:65])
                rc = p2t.tile([128, 1], F32, tag="rc", name="rc")
                nc.vector.reciprocal(rc[:], tq[:, 64:65])
                tb = s * 4 + qb
                nc.vector.tensor_scalar_mul(
                    ysb[tb][:, h * 64:(h + 1) * 64],
                    tq[:, 0:64], rc[:, 0:1])

        def attention_half(s):
            kcmax = 4 * (s + 1)
            for hp in range(4):  # head pairs, 2-deep lookahead
                h0, h1 = 2 * hp, 2 * hp + 1
                yp0 = psB.tile([65, 512], F32, tag="yp0", name="yp0")
                yp1 = psB.tile([65, 512], F32, tag="yp1", name="yp1")
                pts0, pts1 = {}, {}
                for k in range(min(2, kcmax)):
                    scores(s, h0, k, pts0)
                    scores(s, h1, k, pts1)
                for kc in range(kcmax):
                    if kc + 2 < kcmax:
                        scores(s, h0, kc + 2, pts0)
                        scores(s, h1, kc + 2, pts1)
                    pv(s, h0, kc, kcmax, yp0, pts0)
                    pv(s, h1, kc, kcmax, yp1, pts1)
                normalize(s, h0, yp0)
                normalize(s, h1, yp1)

        def outproj_half(s):
            for tb in range(s * 4, s * 4 + 4):
                trow = slice(tb * 128, (tb + 1) * 128)
                for p in range(4):
                    tY = psC.tile([128, 128], F32, tag="tp2", name="tY")
                    nc.tensor.transpose(
                        tY[:], ysb[tb][:, p * 128:(p + 1) * 128], ident[:])
                    if (tb + p) % 2 == 0:
                        nc.vector.tensor_copy(yT[p][:, trow], tY[:])
                    else:
                        nc.scalar.copy(yT[p][:, trow], tY[:])
            for tb in range(s * 4, s * 4 + 4):
                trow = slice(tb * 128, (tb + 1) * 128)
                for n in range(4):
                    op = psA.tile([128, 512], F32, tag="sp", name="op")
                    for p in range(4):
                        nc.tensor.matmul(
                            op[:], yT[p][:, trow],
                            wo[p][:, n * 512:(n + 1) * 512],
                            start=(p == 0), stop=(p == 3))
                    oe = ost.tile([128, 512], F32, tag="oe", name="oe")
                    if n % 2 == 0:
                        nc.scalar.copy(oe[:], op[:])
                    else:
                        nc.vector.tensor_copy(oe[:], op[:])
                    nc.gpsimd.dma_start(
                        out_d[trow, n * 512:(n + 1) * 512], oe[:])

        # s=0 attention overlaps th1 projection below
        attention_half(0)

        # --------- th1 projection: blk-outer on shared psA slots ----------
        for blk in (4, 0, 1, 2, 3, 5):
            pp = psA.tile([128, 512], F32, tag="sp", name=f"pp1_{blk}")
            for ci in range(16):
                if blk < 4:
                    lhsT = wqs[ci][:, blk * 128:(blk + 1) * 128]
                else:
                    lhsT = wkvs[ci][:, (blk - 4) * 128:(blk - 3) * 128]
                nc.tensor.matmul(pp[:], lhsT, xb[ci][:],
                                 start=(ci == 0), stop=(ci == 15))
            if blk == 5:
                vsb1 = v_evac(pp)
            else:
                rope_blk(pp, blk, 1, tab_b)
        v_transposes(1, vsb1, psC)

        p1xb_cm.__exit__(None, None, None)
        p1t_cm.__exit__(None, None, None)
        p1w_cm.__exit__(None, None, None)

        p2c_cm = tc.tile_pool(name="p2c", bufs=1, side="right")
        p2c = p2c_cm.__enter__()
        ost_cm = tc.tile_pool(name="ost", bufs=4, side="right")
        ost = ost_cm.__enter__()
        wo = []
        for p in range(4):
            t = p2c.tile([128, C], F32R, tag=f"wo{p}", name=f"wo{p}")
            nc.scalar.dma_start(t[:], wo_d[p * 128:(p + 1) * 128, :].bitcast(F32R))
            wo.append(t)
        yT = [p2c.tile([128, T], F32R, tag=f"yT{p}", name=f"yTt{p}")
              for p in range(4)]

        outproj_half(0)
        attention_half(1)
        outproj_half(1)

        for cm in (ost_cm, p2c_cm, psC_cm, psB_cm, psA_cm, p1t_cm if False else None,
                   p2t_cm, ptp_cm, p2y_cm, cp_cm):
            if cm is not None:
                cm.__exit__(None, None, None)

    nc.compile()
    return nc


def make_core_inputs(x, cos, sin, attention_mask, is_vision, Wq, Wk, Wv, Wo,
                     gate, b, g):
    cos_b = np.asarray(cos[b], dtype=np.float32)   # [T, 64]
    sin_b = np.asarray(sin[b], dtype=np.float32)
    sgn = np.concatenate([-np.ones(32), np.ones(32)]).astype(np.float32)
    cosT = np.tile(cos_b.T, (2, 1))                            # [128, T]
    sinT = np.tile(sin_b.T * sgn[:, None], (2, 1))             # [128, T]
    vk = np.asarray(is_vision[b], dtype=np.int32)
    maskneg = np.where(np.asarray(attention_mask[b]) > 0, 0.0, NEG)

    hq0 = QH * g
    btab = np.empty((128, 128), dtype=np.float32)
    for h in range(QH):
        for vq in range(2):
            for kc in range(8):
                col = h * 16 + vq * 8 + kc
                ks = slice(kc * 128, (kc + 1) * 128)
                btab[:, col] = gate[hq0 + h, 2 * vq + vk[ks]] + maskneg[ks]

    return {
        "xT": np.ascontiguousarray(x[b].T),
        "wqT": np.ascontiguousarray(Wq[hq0 * 64:hq0 * 64 + 512, :].T),
        "wkvT": np.ascontiguousarray(
            np.concatenate([Wk[128 * g:128 * g + 128, :].T,
                            Wv[128 * g:128 * g + 128, :].T], axis=1)),
        "woT": np.ascontiguousarray(Wo[:, hq0 * 64:hq0 * 64 + 512].T),
        "cosqT": np.ascontiguousarray(cosT * 0.125),
        "sinqT": np.ascontiguousarray(sinT * 0.125),
        "coskT": np.ascontiguousarray(cosT),
        "sinkT": np.ascontiguousarray(sinT),
        "btab": btab,
        "maskT": np.where(np.arange(128)[:, None] <= np.arange(128)[None, :],
                          0.0, NEG).astype(np.float32),
        "ones": np.ones((128, 1), dtype=np.float32),
    }


def kernel(x, cos, sin, attention_mask, is_vision, Wq, Wk, Wv, Wo, gate):
    x = np.asarray(x, dtype=np.float32)
    cos = np.asarray(cos, dtype=np.float32)
    sin = np.asarray(sin, dtype=np.float32)
    attention_mask = np.asarray(attention_mask, dtype=np.float32)
    is_vision = np.asarray(is_vision)
    Wq = np.asarray(Wq, dtype=np.float32)
    Wk = np.asarray(Wk, dtype=np.float32)
    Wv = np.asarray(Wv, dtype=np.float32)
    Wo = np.asarray(Wo, dtype=np.float32)
    gate = np.asarray(gate, dtype=np.float32)

    # q-side vision flag must be constant within each 128-token tile and
    # identical across batches (holds for the fixed vision-prefix data).
    iv = is_vision.astype(np.int32)
    qtile_vq = []
    for qt in range(NTB):
        blk = iv[:, qt * 128:(qt + 1) * 128]
        assert (blk == blk[0, 0]).all(), "is_vision not 128-tile constant"
        qtile_vq.append(int(blk[0, 0]))

    in_maps = [
        make_core_inputs(x, cos, sin, attention_mask, is_vision,
                         Wq, Wk, Wv, Wo, gate, b=c // 4, g=c % 4)
        for c in range(NCORES)
    ]

    nc = build_program(qtile_vq)
    trace = bool(int(os.environ.get("NANOVLM_TRACE", "0")))
    if trace:
        results = _run_traced(nc, in_maps)
    else:
        results = run_bass_kernel_spmd(nc, in_maps, list(range(NCORES))).results
    out = np.empty((B, T, C), dtype=np.float32)
    for b in range(B):
        out[b] = sum(np.asarray(results[4 * b + g]["out"], dtype=np.float32)
                     for g in range(4))
    return out


def _ensure_ntff_hook():
    """The agent image's antenv lacks axon_hooks; shim it and register the
    ctypes NTFF profile hook against the axon PJRT .so."""
    try:
        from antenv.axon_hooks import get_axon_ntff_profile_hook  # noqa: F401
        return True
    except ImportError:
        pass
    import types

    import antenv

    mod = types.ModuleType("antenv.axon_hooks")
    mod._hook = None

    def set_axon_ntff_profile_hook(h):
        mod._hook = h

    def get_axon_ntff_profile_hook():
        return mod._hook

    mod.set_axon_ntff_profile_hook = set_axon_ntff_profile_hook
    mod.get_axon_ntff_profile_hook = get_axon_ntff_profile_hook
    sys.modules["antenv.axon_hooks"] = mod
    antenv.axon_hooks = mod
    if "/root/.axon_site" not in sys.path:
        sys.path.insert(0, "/root/.axon_site")
    try:
        from trn_agent_boot.trn_boot import _ntff_profile_via_ctypes

        hook = _ntff_profile_via_ctypes("/opt/axon/libaxon_pjrt.so")
    except Exception as e:
        print("ntff hook setup failed:", e)
        return False
    if hook is None:
        return False
    set_axon_ntff_profile_hook(hook)
    return True


def _run_traced(nc, in_maps, trace_core=0):
    import glob
    import tempfile

    from concourse import bass2jax
    from concourse._compat import FishPath
    import gauge.profiler

    if not _ensure_ntff_hook():
        print("no NTFF hook; running untraced")
        return run_bass_kernel_spmd(nc, in_maps, list(range(NCORES))).results

    from antenv.axon_hooks import get_axon_ntff_profile_hook

    hook = get_axon_ntff_profile_hook()
    tmpdir = tempfile.mkdtemp(prefix="nanovlm_prof_")
    with hook(tmpdir, [trace_core]):
        results = bass2jax.run_bass_via_pjrt(nc, in_maps, n_cores=NCORES)
    ntffs = glob.glob(os.path.join(tmpdir, "*_body*.ntff"))
    if not ntffs:
        print("no NTFF produced; files:", os.listdir(tmpdir))
        return results
    profile = gauge.profiler.Profile(
        profile_path=FishPath(tmpdir),
        kernel_dev_mode=True,
        profile_on_exit=False,
        bass_kernel=nc.m,
        offline_processing=True,
        fname="*_body*",
    )
    try:
        pr = profile.to_perfetto(model_index=(trace_core,))
        kernel.last_exec_time_ns = pr[0].exec_time_ns
        kernel.last_trace = pr[0].trace_path
        print(f"HW exec time: {pr[0].exec_time_ns} ns")
        print("trace:", pr[0].trace_path)
    except Exception as e:
        print("perfetto conversion failed:", type(e).__name__, e)
        print("ntff dir:", tmpdir)
    return results



# revision 7
# speedup vs baseline: 1.1545x; 1.1545x over previous
"""NanoVLM GQA attention block on 8 Trainium2 NeuronCores.

Sharding: core c = 4*b + g handles batch b (of 2) and head-group g (of 4):
8 q-heads (global 8g..8g+8) and 2 kv-heads (2g, 2g+1). Each core computes a
partial output restricted to its heads' columns of Wo; the host sums the 4
partials per batch (the tensor-parallel reduce, done host-side).

v2: full fp16 datapath (PSUM accumulation stays f32). All PE transposes are
replaced by XBAR DMA transposes (2-byte dtype), the causal diagonal mask is
applied as a post-exp tril multiply on the DVE, rotate-half partition swaps
are batched into 2 DMAs per block, and the output is written as one DMA per
128-token row block. 1/sqrt(64) is folded into Wq on the host so q and k
share one cos/sin table pair.

Device pipeline (per core):
  1. proj with WEIGHTS stationary -> q/k/v d-major [hd, t] in PSUM (fp16
     operands, f32 accum); q as 4 head-pair tiles of 128 partitions
  2. RoPE in d-major: rotate_half = partition 32<->64 block swap via 2
     batched SBUF->SBUF DMAs (sign folded into the sin table); tables f32
  3. v evacuated fp16 and XBAR-DMA-transposed to t-major; vA [128,66]
     slices of one [128,16,66] tile carry an appended ones column
  4. scores k-major: sp [128k, 512q] = kT^T @ qT (fp16), exp on ACT with
     per-partition bias = gate[h,2vq+vk] + log-mask, writing P^T fp16;
     diagonal 128-block gets a post-exp tril multiply on DVE
  5. PV: y_aug [65, 512q] += vA^T @ P^T (row 64 = softmax denom l)
  6. normalize: evac y_aug pair fp16 -> [80,1024], XBAR transpose to
     q-major, reciprocal of col 64, tensor_scalar multiply -> ynorm
     [128q, 512hd-pair], XBAR transpose back into yT [hd, t]
  7. out-proj psum [128t, 512n] over 4 hd-chunks, evac fp16, one DMA per
     t-block writes partial out [1024, 2048] fp16; host sums in f32
"""

import os
import sys

sys.path.insert(0, "/opt/trn_rl_repo")

import numpy as np

import concourse.bacc as bacc
import concourse.mybir as mybir
import concourse.tile as tile
from concourse.bass_utils import run_bass_kernel_spmd

F32 = mybir.dt.float32
F16 = mybir.dt.float16
AF = mybir.ActivationFunctionType
ALU = mybir.AluOpType

B, T, C = 2, 1024, 2048
NH, NKV, HD = 32, 8, 64
QH, KVH = 8, 2          # per-core q-heads / kv-heads
NTB = T // 128          # 8 t-blocks
NCORES = 8
NEG = -1e30


def build_program(qtile_vq):
    """qtile_vq: per 128-token q-tile, the is_vision value (0/1), len 8."""
    nc = bacc.Bacc("TRN2", target_bir_lowering=False, debug=False,
                   num_devices=NCORES)

    xT_d = nc.dram_tensor("xT", [C, T], F16, kind="ExternalInput").ap()
    wq_d = nc.dram_tensor("wqT", [C, 512], F16, kind="ExternalInput").ap()
    wkv_d = nc.dram_tensor("wkvT", [C, 256], F16, kind="ExternalInput").ap()
    wo_d = nc.dram_tensor("woT", [512, C], F16, kind="ExternalInput").ap()
    cos_d = nc.dram_tensor("cosT", [128, T], F32, kind="ExternalInput").ap()
    sin_d = nc.dram_tensor("sinT", [128, T], F32, kind="ExternalInput").ap()
    btab_d = nc.dram_tensor("btab", [128, 128], F32, kind="ExternalInput").ap()
    tril_d = nc.dram_tensor("trilT", [128, 128], F16, kind="ExternalInput").ap()
    ones_d = nc.dram_tensor("ones16", [128, 16], F16, kind="ExternalInput").ap()
    out_d = nc.dram_tensor("out", [T, C], F16, kind="ExternalOutput").ap()

    with tile.TileContext(nc) as tc:
        cp_cm = tc.tile_pool(name="const", bufs=1)
        cp = cp_cm.__enter__()
        btab = cp.tile([128, 128], F32, tag="btab")
        nc.scalar.dma_start(btab[:], btab_d)
        trilT = cp.tile([128, 128], F16, tag="trilT")
        nc.scalar.dma_start(trilT[:], tril_d)
        cosT = cp.tile([128, T], F32, tag="cosT")
        nc.scalar.dma_start(cosT[:], cos_d)
        sinT = cp.tile([128, T], F32, tag="sinT")
        nc.scalar.dma_start(sinT[:], sin_d)
        qTp = [cp.tile([128, T], F16, tag=f"qTp{p}", name=f"qTp{p}")
               for p in range(4)]
        kTp = cp.tile([128, T], F16, tag="kTp")
        kTs = cp.tile([128, T], F16, tag="kTs")  # kv heads swapped
        # [t-part, (tb,j) block, 66]: cols 0:64 v data, col 64 ones
        vAall = cp.tile([128, 16, 66], F16, tag="vAall")
        # v t-major landing pads: per th, per kv-head j: [128t, 4 qb, 64d]
        vT = [cp.tile([128, 4, 64], F16, tag=f"vT{i}", name=f"vT{i}")
              for i in range(4)]
        # y_aug evac pads: rows 0:65 used, 65:80 zero filler for the XBAR
        # transpose; declared [96,T] so the filler memset starts at the
        # 32-aligned partition 64 (row 64 is rewritten by every evac)
        yap = [cp.tile([96, T], F16, tag=f"yap{i}", name=f"yap{i}")
               for i in range(4)]
        yT = [cp.tile([128, T], F16, tag=f"yT{p}", name=f"yTt{p}")
              for p in range(4)]

        # --------- phase-1 pools (th0 x + weights / tmps) -----------------
        p1w_cm = tc.tile_pool(name="p1w", bufs=1, side="right")
        p1w = p1w_cm.__enter__()
        p1t_cm = tc.tile_pool(name="p1t", bufs=2, side="right")
        p1t = p1t_cm.__enter__()
        p1xa_cm = tc.tile_pool(name="p1xa", bufs=1, side="right")
        p1xa = p1xa_cm.__enter__()
        p1ps_cm = tc.tile_pool(name="p1ps", bufs=1, space="PSUM")
        p1ps = p1ps_cm.__enter__()

        xa, wqs, wkvs = [], [], []
        for i in range(16):
            xt = p1xa.tile([128, 512], F16, tag=f"xa{i}", name=f"xa{i}")
            nc.sync.dma_start(xt[:], xT_d[i * 128:(i + 1) * 128, 0:512])
            xa.append(xt)
            wt = p1w.tile([128, 512], F16, tag=f"wq{i}", name=f"wq{i}")
            nc.gpsimd.dma_start(wt[:], wq_d[i * 128:(i + 1) * 128, :])
            wqs.append(wt)
            kt = p1w.tile([128, 256], F16, tag=f"wkv{i}", name=f"wkv{i}")
            nc.scalar.dma_start(kt[:], wkv_d[i * 128:(i + 1) * 128, :])
            wkvs.append(kt)

        # late gpsimd setup (after weight DMA issues): ones col + yap floor
        nc.gpsimd.dma_start(vAall[:, :, 64:65], ones_d)
        for i in range(4):
            nc.gpsimd.memset(yap[i][64:96, :], 0.0)

        def rope_blk(pp, blk, th):
            """pp: [128,512] psum with d-major proj; writes qTp/kTp th-slice."""
            tsl = slice(th * 512, (th + 1) * 512)
            dstT = qTp[blk] if blk < 4 else kTp
            ev = p1t.tile([128, 512], F16, tag="ev", name="ev")
            nc.scalar.copy(ev[:], pp[:])
            rot = p1t.tile([128, 512], F16, tag="rot", name="rot")
            for q0 in (0, 64):
                nc.gpsimd.dma_start(rot[q0:q0 + 32, :], ev[q0 + 32:q0 + 64, :])
                nc.gpsimd.dma_start(rot[q0 + 32:q0 + 64, :], ev[q0:q0 + 32, :])
            t1 = p1t.tile([128, 512], F32, tag="t1", name="t1")
            nc.vector.tensor_mul(t1[:], pp[:], cosT[:, tsl])
            t2 = p1t.tile([128, 512], F32, tag="t2", name="t2")
            nc.vector.tensor_mul(t2[:], rot[:], sinT[:, tsl])
            nc.vector.tensor_add(dstT[:, tsl], t1[:], t2[:])
            if blk == 4:
                nc.gpsimd.dma_start(kTs[0:64, tsl], kTp[64:128, tsl])
                nc.gpsimd.dma_start(kTs[64:128, tsl], kTp[0:64, tsl])

        def v_path(th, pp):
            """evac v proj psum + transpose to t-major vAall slices."""
            vsb = p1t.tile([128, 512], F16, tag="vsb", name="vsb")
            nc.scalar.copy(vsb[:], pp[:])
            for j in range(KVH):
                vt = vT[th * 2 + j]
                nc.sync.dma_start_transpose(vt[:], vsb[j * 64:(j + 1) * 64, :])
                for qb in range(4):
                    tb = th * 4 + qb
                    nc.vector.tensor_copy(vAall[:, tb * 2 + j, 0:64],
                                          vt[:, qb, :])

        # --------- th0 projection: ci-outer over 6 psum blocks ------------
        pps = [p1ps.tile([128, 512], F32, tag=f"pp{b}", name=f"pp{b}")
               for b in range(6)]
        for ci in range(16):
            for blk in (4, 0):
                lhsT = (wqs[ci][:, 0:128] if blk == 0
                        else wkvs[ci][:, 0:128])
                nc.tensor.matmul(pps[blk][:], lhsT, xa[ci][:],
                                 start=(ci == 0), stop=(ci == 15))
        rope_blk(pps[4], 4, 0)
        rope_blk(pps[0], 0, 0)
        for ci in range(16):
            for blk in (1, 2, 3, 5):
                if blk < 4:
                    lhsT = wqs[ci][:, blk * 128:(blk + 1) * 128]
                else:
                    lhsT = wkvs[ci][:, 128:256]
                nc.tensor.matmul(pps[blk][:], lhsT, xa[ci][:],
                                 start=(ci == 0), stop=(ci == 15))
        for blk in (1, 2, 3):
            rope_blk(pps[blk], blk, 0)
        v_path(0, pps[5])

        p1ps_cm.__exit__(None, None, None)
        p1xa_cm.__exit__(None, None, None)

        # --------- attention pools (+ th1 x, DMA'd now) -------------------
        ptp_cm = tc.tile_pool(name="ptp", bufs=8)
        ptp = ptp_cm.__enter__()
        p2t_cm = tc.tile_pool(name="p2t", bufs=4)
        p2t = p2t_cm.__enter__()
        p1xb_cm = tc.tile_pool(name="p1xb", bufs=1, side="right")
        p1xb = p1xb_cm.__enter__()
        psA_cm = tc.tile_pool(name="psA", bufs=6, space="PSUM")
        psA = psA_cm.__enter__()
        psB_cm = tc.tile_pool(name="psB", bufs=1, space="PSUM")
        psB = psB_cm.__enter__()

        xb = []
        for i in range(16):
            xt = p1xb.tile([128, 512], F16, tag=f"xb{i}", name=f"xb{i}")
            nc.sync.dma_start(xt[:], xT_d[i * 128:(i + 1) * 128, 512:1024])
            xb.append(xt)

        def scores(s, h, kc, pts):
            j, p, r = h // 4, h // 2, (h % 2) * 64
            kt = kTp if j * 64 == r else kTs
            ql = max(0, kc * 128 - s * 512)
            sp = psA.tile([128, 512], F32, tag="sp", name="sp")
            nc.tensor.matmul(
                sp[:, ql:512],
                kt[r:r + 64, kc * 128:(kc + 1) * 128],
                qTp[p][r:r + 64, s * 512 + ql:(s + 1) * 512],
                start=True, stop=True)
            pt = ptp.tile([128, 512], F16, tag="pt", name="pt")
            c = ql  # multiple of 128
            while c < 512:
                vq = qtile_vq[s * 4 + c // 128]
                ce = c
                while ce < 512 and qtile_vq[s * 4 + ce // 128] == vq:
                    ce += 128
                col = h * 16 + vq * 8 + kc
                nc.scalar.activation(pt[:, c:ce], sp[:, c:ce], AF.Exp,
                                     bias=btab[:, col:col + 1], scale=1.0)
                c = ce
            if s * 4 <= kc < s * 4 + 4:
                # causal diagonal block: zero upper triangle post-exp
                dc = kc * 128 - s * 512
                nc.vector.tensor_mul(pt[:, dc:dc + 128],
                                     pt[:, dc:dc + 128], trilT[:])
            pts[kc] = pt

        def pv(s, h, kc, kcmax, yp, pts):
            j = h // 4
            ql = max(0, kc * 128 - s * 512)
            nc.tensor.matmul(
                yp[:, ql:512], vAall[:, kc * 2 + j, 0:65], pts[kc][:, ql:512],
                start=(kc == 0), stop=(kc == kcmax - 1),
                skip_group_check=True)
            pts[kc] = None

        def normalize_pair(s, hp, yp0, yp1, ci):
            ya = yap[ci % 4]
            nc.scalar.copy(ya[0:65, 0:512], yp0[:])
            nc.vector.tensor_copy(ya[0:65, 512:1024], yp1[:])
            yaT = p2t.tile([128, 8, 80], F16, tag="yaT", name="yaT")
            nc.sync.dma_start_transpose(yaT[:], ya[0:80, :])
            ynorm = p2t.tile([128, 512], F16, tag="ynorm", name="ynorm")
            for qb in range(4):
                rc0 = p2t.tile([128, 1], F32, tag="rc", name="rc0")
                nc.vector.reciprocal(rc0[:], yaT[:, qb, 64:65])
                nc.vector.tensor_scalar_mul(
                    ynorm[:, qb * 128:qb * 128 + 64],
                    yaT[:, qb, 0:64], rc0[:, 0:1])
                rc1 = p2t.tile([128, 1], F32, tag="rc", name="rc1")
                nc.vector.reciprocal(rc1[:], yaT[:, 4 + qb, 64:65])
                nc.vector.tensor_scalar_mul(
                    ynorm[:, qb * 128 + 64:qb * 128 + 128],
                    yaT[:, 4 + qb, 0:64], rc1[:, 0:1])
            nc.sync.dma_start_transpose(
                yT[hp][:, s * 512:(s + 1) * 512].rearrange(
                    "p (b c) -> p b c", b=4), ynorm[:])

        def attention_half(s):
            kcmax = 4 * (s + 1)
            for hp in range(4):  # head pairs, 2-deep lookahead
                h0, h1 = 2 * hp, 2 * hp + 1
                yp0 = psB.tile([65, 512], F32, tag="yp0", name="yp0")
                yp1 = psB.tile([65, 512], F32, tag="yp1", name="yp1")
                pts0, pts1 = {}, {}
                for k in range(min(2, kcmax)):
                    scores(s, h0, k, pts0)
                    scores(s, h1, k, pts1)
                for kc in range(kcmax):
                    if kc + 2 < kcmax:
                        scores(s, h0, kc + 2, pts0)
                        scores(s, h1, kc + 2, pts1)
                    pv(s, h0, kc, kcmax, yp0, pts0)
                    pv(s, h1, kc, kcmax, yp1, pts1)
                normalize_pair(s, hp, yp0, yp1, s * 4 + hp)

        def outproj_half(s):
            for tb in range(s * 4, s * 4 + 4):
                trow = slice(tb * 128, (tb + 1) * 128)
                oe = ost.tile([128, C], F16, tag="oe", name="oe")
                for n in range(4):
                    op = psA.tile([128, 512], F32, tag="sp", name="op")
                    for p in range(4):
                        nc.tensor.matmul(
                            op[:], yT[p][:, trow],
                            wo[p][:, n * 512:(n + 1) * 512],
                            start=(p == 0), stop=(p == 3))
                    if n % 2 == 0:
                        nc.scalar.copy(oe[:, n * 512:(n + 1) * 512], op[:])
                    else:
                        nc.vector.tensor_copy(oe[:, n * 512:(n + 1) * 512],
                                              op[:])
                nc.gpsimd.dma_start(out_d[trow, :], oe[:])

        # s=0 attention overlaps th1 projection below
        attention_half(0)

        # --------- th1 projection: blk-outer on shared psA slots ----------
        for blk in (4, 0, 1, 2, 3, 5):
            pp = psA.tile([128, 512], F32, tag="sp", name=f"pp1_{blk}")
            for ci in range(16):
                if blk < 4:
                    lhsT = wqs[ci][:, blk * 128:(blk + 1) * 128]
                else:
                    lhsT = wkvs[ci][:, (blk - 4) * 128:(blk - 3) * 128]
                nc.tensor.matmul(pp[:], lhsT, xb[ci][:],
                                 start=(ci == 0), stop=(ci == 15))
            if blk == 5:
                v_path(1, pp)
            else:
                rope_blk(pp, blk, 1)

        p1xb_cm.__exit__(None, None, None)
        p1t_cm.__exit__(None, None, None)
        p1w_cm.__exit__(None, None, None)

        p2c_cm = tc.tile_pool(name="p2c", bufs=1, side="right")
        p2c = p2c_cm.__enter__()
        ost_cm = tc.tile_pool(name="ost", bufs=2, side="right")
        ost = ost_cm.__enter__()
        wo = []
        for p in range(4):
            t = p2c.tile([128, C], F16, tag=f"wo{p}", name=f"wo{p}")
            nc.scalar.dma_start(t[:], wo_d[p * 128:(p + 1) * 128, :])
            wo.append(t)

        outproj_half(0)
        attention_half(1)
        outproj_half(1)

        for cm in (ost_cm, p2c_cm, psB_cm, psA_cm,
                   p2t_cm, ptp_cm, cp_cm):
            cm.__exit__(None, None, None)

    nc.compile()
    return nc


def make_core_inputs(x, cos, sin, attention_mask, is_vision, Wq, Wk, Wv, Wo,
                     gate, b, g):
    cos_b = np.asarray(cos[b], dtype=np.float32)   # [T, 64]
    sin_b = np.asarray(sin[b], dtype=np.float32)
    sgn = np.concatenate([-np.ones(32), np.ones(32)]).astype(np.float32)
    cosT = np.tile(cos_b.T, (2, 1))                            # [128, T]
    sinT = np.tile(sin_b.T * sgn[:, None], (2, 1))             # [128, T]
    vk = np.asarray(is_vision[b], dtype=np.int32)
    maskneg = np.where(np.asarray(attention_mask[b]) > 0, 0.0, NEG)

    hq0 = QH * g
    btab = np.empty((128, 128), dtype=np.float32)
    for h in range(QH):
        for vq in range(2):
            for kc in range(8):
                col = h * 16 + vq * 8 + kc
                ks = slice(kc * 128, (kc + 1) * 128)
                btab[:, col] = gate[hq0 + h, 2 * vq + vk[ks]] + maskneg[ks]

    return {
        "xT": np.ascontiguousarray(x[b].T).astype(np.float16),
        "wqT": np.ascontiguousarray(
            Wq[hq0 * 64:hq0 * 64 + 512, :].T * 0.125).astype(np.float16),
        "wkvT": np.ascontiguousarray(
            np.concatenate([Wk[128 * g:128 * g + 128, :].T,
                            Wv[128 * g:128 * g + 128, :].T],
                           axis=1)).astype(np.float16),
        "woT": np.ascontiguousarray(
            Wo[:, hq0 * 64:hq0 * 64 + 512].T).astype(np.float16),
        "cosT": np.ascontiguousarray(cosT),
        "sinT": np.ascontiguousarray(sinT),
        "btab": btab,
        "trilT": (np.arange(128)[:, None] <= np.arange(128)[None, :]
                  ).astype(np.float16),
        "ones16": np.ones((128, 16), dtype=np.float16),
    }


def kernel(x, cos, sin, attention_mask, is_vision, Wq, Wk, Wv, Wo, gate):
    x = np.asarray(x, dtype=np.float32)
    cos = np.asarray(cos, dtype=np.float32)
    sin = np.asarray(sin, dtype=np.float32)
    attention_mask = np.asarray(attention_mask, dtype=np.float32)
    is_vision = np.asarray(is_vision)
    Wq = np.asarray(Wq, dtype=np.float32)
    Wk = np.asarray(Wk, dtype=np.float32)
    Wv = np.asarray(Wv, dtype=np.float32)
    Wo = np.asarray(Wo, dtype=np.float32)
    gate = np.asarray(gate, dtype=np.float32)

    # q-side vision flag must be constant within each 128-token tile and
    # identical across batches (holds for the fixed vision-prefix data).
    iv = is_vision.astype(np.int32)
    qtile_vq = []
    for qt in range(NTB):
        blk = iv[:, qt * 128:(qt + 1) * 128]
        assert (blk == blk[0, 0]).all(), "is_vision not 128-tile constant"
        qtile_vq.append(int(blk[0, 0]))

    in_maps = [
        make_core_inputs(x, cos, sin, attention_mask, is_vision,
                         Wq, Wk, Wv, Wo, gate, b=c // 4, g=c % 4)
        for c in range(NCORES)
    ]

    nc = build_program(qtile_vq)
    trace = bool(int(os.environ.get("NANOVLM_TRACE", "0")))
    if trace:
        results = _run_traced(nc, in_maps)
    else:
        results = run_bass_kernel_spmd(nc, in_maps, list(range(NCORES))).results
    out = np.empty((B, T, C), dtype=np.float32)
    for b in range(B):
        out[b] = sum(np.asarray(results[4 * b + g]["out"], dtype=np.float32)
                     for g in range(4))
    return out


def _ensure_ntff_hook():
    """The agent image's antenv lacks axon_hooks; shim it and register the
    ctypes NTFF profile hook against the axon PJRT .so."""
    try:
        from antenv.axon_hooks import get_axon_ntff_profile_hook  # noqa: F401
        return True
    except ImportError:
        pass
    import types

    import antenv

    mod = types.ModuleType("antenv.axon_hooks")
    mod._hook = None

    def set_axon_ntff_profile_hook(h):
        mod._hook = h

    def get_axon_ntff_profile_hook():
        return mod._hook

    mod.set_axon_ntff_profile_hook = set_axon_ntff_profile_hook
    mod.get_axon_ntff_profile_hook = get_axon_ntff_profile_hook
    sys.modules["antenv.axon_hooks"] = mod
    antenv.axon_hooks = mod
    if "/root/.axon_site" not in sys.path:
        sys.path.insert(0, "/root/.axon_site")
    try:
        from trn_agent_boot.trn_boot import _ntff_profile_via_ctypes

        hook = _ntff_profile_via_ctypes("/opt/axon/libaxon_pjrt.so")
    except Exception as e:
        print("ntff hook setup failed:", e)
        return False
    if hook is None:
        return False
    set_axon_ntff_profile_hook(hook)
    return True


def _run_traced(nc, in_maps, trace_core=0):
    import glob
    import tempfile

    from concourse import bass2jax
    from concourse._compat import FishPath
    import gauge.profiler

    if not _ensure_ntff_hook():
        print("no NTFF hook; running untraced")
        return run_bass_kernel_spmd(nc, in_maps, list(range(NCORES))).results

    from antenv.axon_hooks import get_axon_ntff_profile_hook

    hook = get_axon_ntff_profile_hook()
    tmpdir = tempfile.mkdtemp(prefix="nanovlm_prof_")
    with hook(tmpdir, [trace_core]):
        results = bass2jax.run_bass_via_pjrt(nc, in_maps, n_cores=NCORES)
    ntffs = glob.glob(os.path.join(tmpdir, "*_body*.ntff"))
    if not ntffs:
        print("no NTFF produced; files:", os.listdir(tmpdir))
        return results
    profile = gauge.profiler.Profile(
        profile_path=FishPath(tmpdir),
        kernel_dev_mode=True,
        profile_on_exit=False,
        bass_kernel=nc.m,
        offline_processing=True,
        fname="*_body*",
    )
    try:
        pr = profile.to_perfetto(model_index=(trace_core,))
        kernel.last_exec_time_ns = pr[0].exec_time_ns
        kernel.last_trace = pr[0].trace_path
        print(f"HW exec time: {pr[0].exec_time_ns} ns")
        print("trace:", pr[0].trace_path)
    except Exception as e:
        print("perfetto conversion failed:", type(e).__name__, e)
        print("ntff dir:", tmpdir)
    return results


# revision 8
# speedup vs baseline: 1.2396x; 1.0737x over previous
"""NanoVLM GQA attention block on 8 Trainium2 NeuronCores.

Sharding: core c = 4*b + g handles batch b (of 2) and head-group g (of 4):
8 q-heads (global 8g..8g+8) and 2 kv-heads (2g, 2g+1). Each core computes a
partial output restricted to its heads' columns of Wo; the host sums the 4
partials per batch (the tensor-parallel reduce, done host-side).

v3: 16-bit datapath (PSUM accumulation stays f32): fp16 for x/weights/q/k
(matmul precision), bf16 for everything the ACT engine writes (exp output,
psum evacuations — bf16 stores are measurably faster than fp16 on ACT) and
downstream (P, V, y, Wo). All PE transposes are XBAR DMA transposes. The
causal diagonal mask is a post-exp tril multiply on DVE. 1/sqrt(64) is
folded into Wq host-side so q/k share one cos/sin table.

The emission order interleaves work so the in-order PE queue never starves
behind exp (ACT) chains:
  th0 proj (block-sequential, rope chases each block)
  -> [attn(0) pair p | th1 proj block] interleaved
  -> [attn(1) pair p | out-proj t-block of half 0] interleaved
  -> out-proj t-blocks of half 1
All PSUM comes from one shared pool (6 rotating 'sp' banks + 2 pinned
y-accumulator banks) so phase transitions carry no bank-reuse barriers.
"""

import os
import sys

sys.path.insert(0, "/opt/trn_rl_repo")

import numpy as np
import ml_dtypes

import concourse.bacc as bacc
import concourse.mybir as mybir
import concourse.tile as tile
from concourse.bass_utils import run_bass_kernel_spmd

F32 = mybir.dt.float32
F16 = mybir.dt.float16
BF16 = mybir.dt.bfloat16
AF = mybir.ActivationFunctionType
ALU = mybir.AluOpType
BF = ml_dtypes.bfloat16

B, T, C = 2, 1024, 2048
NH, NKV, HD = 32, 8, 64
QH, KVH = 8, 2          # per-core q-heads / kv-heads
NTB = T // 128          # 8 t-blocks
NCORES = 8
NEG = -1e30


def build_program(qtile_vq):
    """qtile_vq: per 128-token q-tile, the is_vision value (0/1), len 8."""
    nc = bacc.Bacc("TRN2", target_bir_lowering=False, debug=False,
                   num_devices=NCORES)

    xT_d = nc.dram_tensor("xT", [C, T], F16, kind="ExternalInput").ap()
    wq_d = nc.dram_tensor("wqT", [C, 512], F16, kind="ExternalInput").ap()
    wkv_d = nc.dram_tensor("wkvT", [C, 256], F16, kind="ExternalInput").ap()
    wo_d = nc.dram_tensor("woT", [512, C], BF16, kind="ExternalInput").ap()
    cos_d = nc.dram_tensor("cosT", [128, T], F32, kind="ExternalInput").ap()
    sin_d = nc.dram_tensor("sinT", [128, T], F32, kind="ExternalInput").ap()
    btab_d = nc.dram_tensor("btab", [128, 128], F32, kind="ExternalInput").ap()
    tril_d = nc.dram_tensor("trilT", [128, 128], BF16, kind="ExternalInput").ap()
    ones_d = nc.dram_tensor("ones16", [128, 16], BF16, kind="ExternalInput").ap()
    out_d = nc.dram_tensor("out", [T, C], BF16, kind="ExternalOutput").ap()

    with tile.TileContext(nc) as tc:
        cp_cm = tc.tile_pool(name="const", bufs=1)
        cp = cp_cm.__enter__()
        qTp = [cp.tile([128, T], F16, tag=f"qTp{p}", name=f"qTp{p}")
               for p in range(4)]
        kTp = cp.tile([128, T], F16, tag="kTp")
        kTs = cp.tile([128, T], F16, tag="kTs")  # kv heads swapped
        # [t-part, (tb,j) block, 66]: cols 0:64 v data, col 64 ones
        vAall = cp.tile([128, 16, 66], BF16, tag="vAall")
        # v t-major landing pads: per th, per kv-head j: [128t, 4 qb, 64d]
        vT = [cp.tile([128, 4, 64], BF16, tag=f"vT{i}", name=f"vT{i}")
              for i in range(4)]
        # y_aug evac pads: rows 0:65 used, 65:80 zero filler for the XBAR
        # transpose; declared [96,T] so the filler memset starts at the
        # 32-aligned partition 64 (row 64 is rewritten by every evac)
        yap = [cp.tile([96, T], BF16, tag=f"yap{i}", name=f"yap{i}")
               for i in range(4)]
        yT = [cp.tile([128, T], BF16, tag=f"yT{p}", name=f"yTt{p}")
              for p in range(4)]
        btab = cp.tile([128, 128], F32, tag="btab")
        trilT = cp.tile([128, 128], BF16, tag="trilT")
        cosT = cp.tile([128, T], F32, tag="cosT")
        sinT = cp.tile([128, T], F32, tag="sinT")

        # --------- pools ---------------------------------------------------
        p1w_cm = tc.tile_pool(name="p1w", bufs=1, side="right")
        p1w = p1w_cm.__enter__()
        p1t_cm = tc.tile_pool(name="p1t", bufs=2, side="right")
        p1t = p1t_cm.__enter__()
        p1x_cm = tc.tile_pool(name="p1x", bufs=1, side="right")
        p1x = p1x_cm.__enter__()
        ps_cm = tc.tile_pool(name="ps", bufs=1, space="PSUM")
        ps = ps_cm.__enter__()
        ptp_cm = tc.tile_pool(name="ptp", bufs=8)
        ptp = ptp_cm.__enter__()
        p2t_cm = tc.tile_pool(name="p2t", bufs=4)
        p2t = p2t_cm.__enter__()
        ost_cm = tc.tile_pool(name="ost", bufs=2, side="right")
        ost = ost_cm.__enter__()

        # --------- input DMAs (queue order matters per engine) -------------
        # scalar ring: wkv (paces first proj block), tables, wo
        # sync ring:   xa then xb (pace the two proj halves)
        # gpsimd ring: wq, then one-time setup
        xa, xb, wqs, wkvs = [], [], [], []
        for i in range(16):
            kt = p1w.tile([128, 256], F16, tag=f"wkv{i}", name=f"wkv{i}")
            nc.scalar.dma_start(kt[:], wkv_d[i * 128:(i + 1) * 128, :])
            wkvs.append(kt)
            xt = p1x.tile([128, 512], F16, tag=f"xa{i}", name=f"xa{i}")
            nc.sync.dma_start(xt[:], xT_d[i * 128:(i + 1) * 128, 0:512])
            xa.append(xt)
            wt = p1w.tile([128, 512], F16, tag=f"wq{i}", name=f"wq{i}")
            nc.gpsimd.dma_start(wt[:], wq_d[i * 128:(i + 1) * 128, :])
            wqs.append(wt)
        nc.scalar.dma_start(cosT[:], cos_d)
        nc.scalar.dma_start(sinT[:], sin_d)
        nc.scalar.dma_start(btab[:], btab_d)
        nc.scalar.dma_start(trilT[:], tril_d)
        wo = []
        for p in range(4):
            t = p1w.tile([128, C], BF16, tag=f"wo{p}", name=f"wo{p}")
            nc.scalar.dma_start(t[:], wo_d[p * 128:(p + 1) * 128, :])
            wo.append(t)
        for i in range(16):
            xt = p1x.tile([128, 512], F16, tag=f"xb{i}", name=f"xb{i}")
            nc.sync.dma_start(xt[:], xT_d[i * 128:(i + 1) * 128, 512:1024])
            xb.append(xt)
        # late gpsimd setup (after weight DMA issues): ones col + yap floor
        nc.gpsimd.dma_start(vAall[:, :, 64:65], ones_d)
        for i in range(4):
            nc.gpsimd.memset(yap[i][64:96, :], 0.0)

        def rope_blk(pp, blk, th):
            """pp: [128,512] psum with d-major proj; writes qTp/kTp th-slice."""
            tsl = slice(th * 512, (th + 1) * 512)
            dstT = qTp[blk] if blk < 4 else kTp
            ev = p1t.tile([128, 512], BF16, tag="ev", name="ev")
            nc.scalar.copy(ev[:], pp[:])
            rot = p1t.tile([128, 512], BF16, tag="rot", name="rot")
            for q0 in (0, 64):
                nc.gpsimd.dma_start(rot[q0:q0 + 32, :], ev[q0 + 32:q0 + 64, :])
                nc.gpsimd.dma_start(rot[q0 + 32:q0 + 64, :], ev[q0:q0 + 32, :])
            t1 = p1t.tile([128, 512], F32, tag="t1", name="t1")
            nc.vector.tensor_mul(t1[:], pp[:], cosT[:, tsl])
            t2 = p1t.tile([128, 512], F32, tag="t2", name="t2")
            nc.vector.tensor_mul(t2[:], rot[:], sinT[:, tsl])
            nc.vector.tensor_add(dstT[:, tsl], t1[:], t2[:])
            if blk == 4:
                nc.gpsimd.dma_start(kTs[0:64, tsl], kTp[64:128, tsl])
                nc.gpsimd.dma_start(kTs[64:128, tsl], kTp[0:64, tsl])

        def v_path(th, pp):
            """evac v proj psum + transpose to t-major vAall slices."""
            vsb = p1t.tile([128, 512], BF16, tag="vsb", name="vsb")
            nc.scalar.copy(vsb[:], pp[:])
            for j in range(KVH):
                vt = vT[th * 2 + j]
                nc.sync.dma_start_transpose(vt[:], vsb[j * 64:(j + 1) * 64, :])
                for qb in range(4):
                    tb = th * 4 + qb
                    nc.vector.tensor_copy(vAall[:, tb * 2 + j, 0:64],
                                          vt[:, qb, :])

        def proj_block(blk, th):
            """one 16-ci accumulation into a rotating sp bank + rope/v."""
            xs = xa if th == 0 else xb
            pp = ps.tile([128, 512], F32, tag="sp", bufs=6,
                         name=f"pp{th}_{blk}")
            for ci in range(16):
                if blk < 4:
                    lhsT = wqs[ci][:, blk * 128:(blk + 1) * 128]
                else:
                    lhsT = wkvs[ci][:, (blk - 4) * 128:(blk - 3) * 128]
                nc.tensor.matmul(pp[:], lhsT, xs[ci][:],
                                 start=(ci == 0), stop=(ci == 15))
            if blk == 5:
                v_path(th, pp)
            else:
                rope_blk(pp, blk, th)

        def scores(s, h, kc, pts):
            j, p, r = h // 4, h // 2, (h % 2) * 64
            kt = kTp if j * 64 == r else kTs
            ql = max(0, kc * 128 - s * 512)
            sp = ps.tile([128, 512], F32, tag="sp", bufs=6, name="sp")
            nc.tensor.matmul(
                sp[:, ql:512],
                kt[r:r + 64, kc * 128:(kc + 1) * 128],
                qTp[p][r:r + 64, s * 512 + ql:(s + 1) * 512],
                start=True, stop=True)
            pt = ptp.tile([128, 512], BF16, tag="pt", name="pt")
            c = ql  # multiple of 128
            while c < 512:
                vq = qtile_vq[s * 4 + c // 128]
                ce = c
                while ce < 512 and qtile_vq[s * 4 + ce // 128] == vq:
                    ce += 128
                col = h * 16 + vq * 8 + kc
                nc.scalar.activation(pt[:, c:ce], sp[:, c:ce], AF.Exp,
                                     bias=btab[:, col:col + 1], scale=1.0)
                c = ce
            if s * 4 <= kc < s * 4 + 4:
                # causal diagonal block: zero upper triangle post-exp
                dc = kc * 128 - s * 512
                nc.vector.tensor_mul(pt[:, dc:dc + 128],
                                     pt[:, dc:dc + 128], trilT[:])
            pts[kc] = pt

        def pv(s, h, kc, kcmax, yp, pts):
            j = h // 4
            ql = max(0, kc * 128 - s * 512)
            nc.tensor.matmul(
                yp[:, ql:512], vAall[:, kc * 2 + j, 0:65], pts[kc][:, ql:512],
                start=(kc == 0), stop=(kc == kcmax - 1),
                skip_group_check=True)
            pts[kc] = None

        def normalize_pair(s, hp, yp0, yp1, ci):
            ya = yap[ci % 4]
            nc.scalar.copy(ya[0:65, 0:512], yp0[:])
            nc.vector.tensor_copy(ya[0:65, 512:1024], yp1[:])
            yaT = p2t.tile([128, 8, 80], BF16, tag="yaT", name="yaT")
            nc.sync.dma_start_transpose(yaT[:], ya[0:80, :])
            ynorm = p2t.tile([128, 512], BF16, tag="ynorm", name="ynorm")
            for qb in range(4):
                rc0 = p2t.tile([128, 1], F32, tag="rc", name="rc0")
                nc.vector.reciprocal(rc0[:], yaT[:, qb, 64:65])
                nc.vector.tensor_scalar_mul(
                    ynorm[:, qb * 128:qb * 128 + 64],
                    yaT[:, qb, 0:64], rc0[:, 0:1])
                rc1 = p2t.tile([128, 1], F32, tag="rc", name="rc1")
                nc.vector.reciprocal(rc1[:], yaT[:, 4 + qb, 64:65])
                nc.vector.tensor_scalar_mul(
                    ynorm[:, qb * 128 + 64:qb * 128 + 128],
                    yaT[:, 4 + qb, 0:64], rc1[:, 0:1])
            nc.sync.dma_start_transpose(
                yT[hp][:, s * 512:(s + 1) * 512].rearrange(
                    "p (b c) -> p b c", b=4), ynorm[:])

        def attention_pair(s, hp):
            kcmax = 4 * (s + 1)
            h0, h1 = 2 * hp, 2 * hp + 1
            yp0 = ps.tile([65, 512], F32, tag="yp0", bufs=1, name="yp0")
            yp1 = ps.tile([65, 512], F32, tag="yp1", bufs=1, name="yp1")
            pts0, pts1 = {}, {}
            for k in range(min(2, kcmax)):
                scores(s, h0, k, pts0)
                scores(s, h1, k, pts1)
            for kc in range(kcmax):
                if kc + 2 < kcmax:
                    scores(s, h0, kc + 2, pts0)
                    scores(s, h1, kc + 2, pts1)
                pv(s, h0, kc, kcmax, yp0, pts0)
                pv(s, h1, kc, kcmax, yp1, pts1)
            normalize_pair(s, hp, yp0, yp1, s * 4 + hp)

        def outproj_tb(tb):
            trow = slice(tb * 128, (tb + 1) * 128)
            oe = ost.tile([128, C], BF16, tag="oe", name="oe")
            for n in range(4):
                op = ps.tile([128, 512], F32, tag="sp", bufs=6, name="op")
                for p in range(4):
                    nc.tensor.matmul(
                        op[:], yT[p][:, trow],
                        wo[p][:, n * 512:(n + 1) * 512],
                        start=(p == 0), stop=(p == 3))
                if n % 2 == 0:
                    nc.scalar.copy(oe[:, n * 512:(n + 1) * 512], op[:])
                else:
                    nc.vector.tensor_copy(oe[:, n * 512:(n + 1) * 512],
                                          op[:])
            nc.sync.dma_start(out_d[trow, 0:1024], oe[:, 0:1024])
            nc.scalar.dma_start(out_d[trow, 1024:2048], oe[:, 1024:2048])

        # --------- emission: th0 proj, then interleaved phases -------------
        for blk in (4, 0, 1, 2, 3, 5):
            proj_block(blk, 0)

        th1_blocks = [4, 0, 1, 2, 3, 5]
        for hp in range(4):  # attn(0) pairs fill ACT; th1 proj fills PE
            attention_pair(0, hp)
            proj_block(th1_blocks[hp], 1)
        for blk in th1_blocks[4:]:
            proj_block(blk, 1)

        for hp in range(4):  # attn(1) pairs fill ACT; outproj(0) fills PE
            attention_pair(1, hp)
            outproj_tb(hp)
        for tb in range(4, 8):
            outproj_tb(tb)

        for cm in (ost_cm, p2t_cm, ptp_cm, ps_cm, p1x_cm, p1t_cm, p1w_cm,
                   cp_cm):
            cm.__exit__(None, None, None)

    nc.compile()
    return nc


def make_core_inputs(x, cos, sin, attention_mask, is_vision, Wq, Wk, Wv, Wo,
                     gate, b, g):
    cos_b = np.asarray(cos[b], dtype=np.float32)   # [T, 64]
    sin_b = np.asarray(sin[b], dtype=np.float32)
    sgn = np.concatenate([-np.ones(32), np.ones(32)]).astype(np.float32)
    cosT = np.tile(cos_b.T, (2, 1))                            # [128, T]
    sinT = np.tile(sin_b.T * sgn[:, None], (2, 1))             # [128, T]
    vk = np.asarray(is_vision[b], dtype=np.int32)
    maskneg = np.where(np.asarray(attention_mask[b]) > 0, 0.0, NEG)

    hq0 = QH * g
    btab = np.empty((128, 128), dtype=np.float32)
    for h in range(QH):
        for vq in range(2):
            for kc in range(8):
                col = h * 16 + vq * 8 + kc
                ks = slice(kc * 128, (kc + 1) * 128)
                btab[:, col] = gate[hq0 + h, 2 * vq + vk[ks]] + maskneg[ks]

    return {
        "xT": np.ascontiguousarray(x[b].T).astype(np.float16),
        "wqT": np.ascontiguousarray(
            Wq[hq0 * 64:hq0 * 64 + 512, :].T * 0.125).astype(np.float16),
        "wkvT": np.ascontiguousarray(
            np.concatenate([Wk[128 * g:128 * g + 128, :].T,
                            Wv[128 * g:128 * g + 128, :].T],
                           axis=1)).astype(np.float16),
        "woT": np.ascontiguousarray(
            Wo[:, hq0 * 64:hq0 * 64 + 512].T).astype(BF),
        "cosT": np.ascontiguousarray(cosT),
        "sinT": np.ascontiguousarray(sinT),
        "btab": btab,
        "trilT": (np.arange(128)[:, None] <= np.arange(128)[None, :]
                  ).astype(BF),
        "ones16": np.ones((128, 16), dtype=BF),
    }


def kernel(x, cos, sin, attention_mask, is_vision, Wq, Wk, Wv, Wo, gate):
    x = np.asarray(x, dtype=np.float32)
    cos = np.asarray(cos, dtype=np.float32)
    sin = np.asarray(sin, dtype=np.float32)
    attention_mask = np.asarray(attention_mask, dtype=np.float32)
    is_vision = np.asarray(is_vision)
    Wq = np.asarray(Wq, dtype=np.float32)
    Wk = np.asarray(Wk, dtype=np.float32)
    Wv = np.asarray(Wv, dtype=np.float32)
    Wo = np.asarray(Wo, dtype=np.float32)
    gate = np.asarray(gate, dtype=np.float32)

    # q-side vision flag must be constant within each 128-token tile and
    # identical across batches (holds for the fixed vision-prefix data).
    iv = is_vision.astype(np.int32)
    qtile_vq = []
    for qt in range(NTB):
        blk = iv[:, qt * 128:(qt + 1) * 128]
        assert (blk == blk[0, 0]).all(), "is_vision not 128-tile constant"
        qtile_vq.append(int(blk[0, 0]))

    in_maps = [
        make_core_inputs(x, cos, sin, attention_mask, is_vision,
                         Wq, Wk, Wv, Wo, gate, b=c // 4, g=c % 4)
        for c in range(NCORES)
    ]

    nc = build_program(qtile_vq)
    trace = bool(int(os.environ.get("NANOVLM_TRACE", "0")))
    if trace:
        results = _run_traced(nc, in_maps)
    else:
        results = run_bass_kernel_spmd(nc, in_maps, list(range(NCORES))).results
    out = np.empty((B, T, C), dtype=np.float32)
    for b in range(B):
        out[b] = sum(np.asarray(results[4 * b + g]["out"], dtype=np.float32)
                     for g in range(4))
    return out


def _ensure_ntff_hook():
    """The agent image's antenv lacks axon_hooks; shim it and register the
    ctypes NTFF profile hook against the axon PJRT .so."""
    try:
        from antenv.axon_hooks import get_axon_ntff_profile_hook  # noqa: F401
        return True
    except ImportError:
        pass
    import types

    import antenv

    mod = types.ModuleType("antenv.axon_hooks")
    mod._hook = None

    def set_axon_ntff_profile_hook(h):
        mod._hook = h

    def get_axon_ntff_profile_hook():
        return mod._hook

    mod.set_axon_ntff_profile_hook = set_axon_ntff_profile_hook
    mod.get_axon_ntff_profile_hook = get_axon_ntff_profile_hook
    sys.modules["antenv.axon_hooks"] = mod
    antenv.axon_hooks = mod
    if "/root/.axon_site" not in sys.path:
        sys.path.insert(0, "/root/.axon_site")
    try:
        from trn_agent_boot.trn_boot import _ntff_profile_via_ctypes

        hook = _ntff_profile_via_ctypes("/opt/axon/libaxon_pjrt.so")
    except Exception as e:
        print("ntff hook setup failed:", e)
        return False
    if hook is None:
        return False
    set_axon_ntff_profile_hook(hook)
    return True


def _run_traced(nc, in_maps, trace_core=0):
    import glob
    import tempfile

    from concourse import bass2jax
    from concourse._compat import FishPath
    import gauge.profiler

    if not _ensure_ntff_hook():
        print("no NTFF hook; running untraced")
        return run_bass_kernel_spmd(nc, in_maps, list(range(NCORES))).results

    from antenv.axon_hooks import get_axon_ntff_profile_hook

    hook = get_axon_ntff_profile_hook()
    tmpdir = tempfile.mkdtemp(prefix="nanovlm_prof_")
    with hook(tmpdir, [trace_core]):
        results = bass2jax.run_bass_via_pjrt(nc, in_maps, n_cores=NCORES)
    ntffs = glob.glob(os.path.join(tmpdir, "*_body*.ntff"))
    if not ntffs:
        print("no NTFF produced; files:", os.listdir(tmpdir))
        return results
    profile = gauge.profiler.Profile(
        profile_path=FishPath(tmpdir),
        kernel_dev_mode=True,
        profile_on_exit=False,
        bass_kernel=nc.m,
        offline_processing=True,
        fname="*_body*",
    )
    try:
        pr = profile.to_perfetto(model_index=(trace_core,))
        kernel.last_exec_time_ns = pr[0].exec_time_ns
        kernel.last_trace = pr[0].trace_path
        print(f"HW exec time: {pr[0].exec_time_ns} ns")
        print("trace:", pr[0].trace_path)
    except Exception as e:
        print("perfetto conversion failed:", type(e).__name__, e)
        print("ntff dir:", tmpdir)
    return results
